# revision 24
# baseline (speedup 1.0000x reference)
"""CrossFormer layer (nn_CrossFormerLayer) on 8 trn2 NeuronCores.

Sharding: core c -> batch b = c//2, i-half ih = c%2 (1024 of the 2048
n1 rows). The x-path is fully local to a core; the y-path needs one
pair-wise (2-core) AllReduce of the partial out2 = attn^T @ v1 (the
contraction over i is split across the pair). The y tail
(out2 @ Wo2 + FFN) is duplicated inside each pair (cheap).

All activations are kept in transposed [feature, seq] layout so every
matmul chains without activation transposes. The attention matrix E
(= exp(logits), softmax numerator) is the one tensor needed in both
orientations; it is PE-transposed per 128x128 block. Softmax skips the
max-subtraction (logits are O(1) by construction). The Transformer-XL
relative shift is free: rel is stored [j, m] with row pitch 1535 in
DRAM and the shifted matrix is a strided read
shifted[j, i] = flat[j*1534 + 511 + i].
"""
import math
import ml_dtypes
import numpy as np

import concourse.bacc as bacc
import concourse.bass as bass
import concourse.mybir as mybir
import concourse.tile as tile
from concourse import bass_utils

f32 = mybir.dt.float32
f32r = mybir.dt.float32r
bf16 = mybir.dt.bfloat16
AF = mybir.ActivationFunctionType
ALU = mybir.AluOpType

B, N1, N2 = 4, 2048, 512
D, H, DK, DV, NRP = 768, 8, 64, 96, 96
DY0 = 1536
DFF = 2 * D
NI = N1 // 2              # 1024 i-rows per core
LREL = N2 + NI            # 1536 rel columns (local m window, padded even)
EPS = 1e-5
SCALE = DK ** -0.5
NCORES = 8
CD = D // 128             # 6
CY = DY0 // 128           # 12
CHD = (H * DK) // 128     # 4
CF = DFF // 128           # 12
JT = N2 // 128            # 4
IT = NI // 128            # 8


def _positional_embed():
    """Enformer relative positional features, pure numpy (fp64->fp32)."""
    n1, fs = N1, NRP
    d = np.arange(-n1 + 1, n1)
    ad = np.abs(d).astype(np.float64)[:, None]
    nb = fs // 6
    max_range = math.log(n1) / math.log(2.0)
    half_life = 2.0 ** np.linspace(3.0, max_range, nb)
    f_exp = np.exp(-math.log(2.0) / half_life * ad)
    cw = 2.0 ** np.arange(1, nb + 1).astype(np.float64) - 1.0
    f_cm = (cw > ad).astype(np.float64)
    stddev = n1 / (2.0 * nb)
    start_mean = n1 / nb
    mean = np.linspace(start_mean, float(n1), nb)
    conc = (mean / stddev) ** 2
    rate = mean / (stddev ** 2)
    lgamma = np.array([math.lgamma(c) for c in conc])
    ad_safe = np.where(ad > 0, ad, 1.0)
    logp = (conc - 1.0) * np.log(ad_safe) - rate * ad - (lgamma - conc * np.log(rate))
    logp = np.where(ad > 0, logp, -np.inf)
    prob = np.exp(logp) + 1e-8
    f_g = prob / np.max(prob, -1, keepdims=True)
    emb = np.concatenate([f_exp, f_cm, f_g], -1)
    sign = np.sign(d).astype(np.float64)[:, None]
    return np.concatenate([emb, sign * emb], -1).astype(np.float32)  # [4095, 96]


def _nsplits(n):
    out, o = [], 0
    while o < n:
        s = min(512, n - o)
        out.append((o, s))
        o += s
    return out


def build_bass():
    nc = bacc.Bacc("TRN2", target_bir_lowering=False, debug=False,
                   num_devices=NCORES)

    F32R_INPUTS = {"xT", "y0T", "W_res"}
    BF16_INPUTS = {"posT", "Wq", "Wk", "Wv1", "Wv2", "Wo1", "Wo2", "Wrel",
                   "fx_w1", "fx_w2", "fy_w1", "fy_w2"}

    def din(name, shape):
        dt = (f32r if name in F32R_INPUTS
              else bf16 if name in BF16_INPUTS else f32)
        return nc.dram_tensor(name, shape, dt, kind="ExternalInput")

    d_in = {}
    for nm, shape in [
        ("xT", [D, NI]), ("y0T", [DY0, N2]), ("posT", [NRP, LREL]),
        ("W_res", [DY0, D]), ("Wq", [D, H * DK]), ("Wk", [D, H * DK]),
        ("Wv1", [D, H * DV]), ("Wv2", [D, H * DV]),
        ("Wo1", [H * DV, D]), ("Wo2", [H * DV, D]), ("Wrel", [NRP, H * DK]),
        ("fx_w1", [D, DFF]), ("fx_w2", [DFF, D]),
        ("fy_w1", [D, DFF]), ("fy_w2", [DFF, D]),
        ("lnx_g", [D, 1]), ("lnx_b", [D, 1]), ("lny_g", [D, 1]),
        ("lny_b", [D, 1]), ("fx_g", [D, 1]), ("fx_b", [D, 1]),
        ("fy_g", [D, 1]), ("fy_b", [D, 1]), ("bo1", [D, 1]), ("bo2", [D, 1]),
        ("fx_b2", [D, 1]), ("fy_b2", [D, 1]), ("fx_b1", [DFF, 1]),
        ("fy_b1", [DFF, 1]), ("relb", [H * DK, 1]),
    ]:
        d_in[nm] = din(nm, shape)
    x5T_d = nc.dram_tensor("x5T", [D, NI], f32, kind="ExternalOutput")
    y5T_d = nc.dram_tensor("y5T", [D, N2], f32, kind="ExternalOutput")

    with tile.TileContext(nc) as tc:
        _build(nc, tc, d_in, x5T_d, y5T_d)
    nc.compile()
    return nc


def _build(nc, tc, d_in, x5T_d, y5T_d):
    def mm(ps_ap, pairs):
        n = len(pairs)
        for i, (l, r) in enumerate(pairs):
            nc.tensor.matmul(ps_ap, l, r, start=(i == 0), stop=(i == n - 1))

    def chunked(pool, name, tag=None):
        d = d_in[name]
        rows, cols = d.shape
        c = (rows + 127) // 128
        t = pool.tile([128, c, cols], d.dtype, tag=(tag or name),
                      name=(tag or name))
        nc.sync.dma_start(t[:], d.ap().rearrange("(c p) n -> p c n", p=128))
        return t

    ccopy = nc.scalar.copy

    with tc.tile_pool(name="const", bufs=1) as cpool, \
         tc.tile_pool(name="dram", bufs=1, space="DRAM") as dram:

        stage32 = cpool.tile([128, 128], f32)
        ones128 = cpool.tile([128, 128], f32r)
        nc.vector.memset(stage32[:], 1.0)
        ccopy(ones128[:], stage32[:])
        ones_bf = cpool.tile([128, 1], bf16)
        ccopy(ones_bf[:], stage32[:, 0:1])
        epst = cpool.tile([128, 1], f32)
        nc.vector.memset(epst[:], EPS)
        vt = {nm: chunked(cpool, nm) for nm in
              ["lnx_g", "lnx_b", "lny_g", "lny_b", "fx_g", "fx_b", "fy_g",
               "fy_b", "bo1", "bo2", "fx_b2", "fy_b2", "fx_b1", "fy_b1",
               "relb"]}

        ydram = dram.tile([D * N2], f32r, name="ydram")
        o1dram = dram.tile([H * DV * NI], bf16, name="o1dram")

        def stats_of(tT, C, N, pspool, spool, tag):
            """LN stats over the feature dim (C*128). The all-ones [128,128]
            lhsT replicates column sums to every partition, so the stats come
            out already broadcast: returns (negmu_b, rstd_b) [128, N]."""
            nfeat = C * 128
            negmu_b = spool.tile([128, N], f32, tag=tag + "_mub",
                                 name=tag + "_mub")
            rstd_b = spool.tile([128, N], f32, tag=tag + "_rsb",
                                name=tag + "_rsb")
            tmp = spool.tile([128, N], f32, tag=tag + "_tmp",
                             name=tag + "_tmp")
            for (o, s) in _nsplits(N):
                ps_s = pspool.tile([128, 512], f32, tag="stat_ps",
                                   name="stat_ps")
                mm(ps_s[:, :s],
                   [(ones128[:], tT[:, ci, o:o + s]) for ci in range(C)])
                nc.scalar.mul(negmu_b[:, o:o + s], ps_s[:, :s], -1.0 / nfeat)
                ps_q = pspool.tile([128, 512], f32, tag="stat_ps",
                                   name="stat_ps")
                for ci in range(C):
                    sq = spool.tile([128, 512], f32r, tag=tag + "_sqb",
                                    name=tag + "_sqb")
                    nc.scalar.square(sq[:, :s], tT[:, ci, o:o + s])
                    nc.tensor.matmul(ps_q[:, :s], ones128[:], sq[:, :s],
                                     start=(ci == 0), stop=(ci == C - 1))
                nc.scalar.mul(tmp[:, o:o + s], ps_q[:, :s], 1.0 / nfeat)
            nc.vector.tensor_mul(rstd_b[:], negmu_b[:], negmu_b[:])
            nc.vector.tensor_sub(tmp[:], tmp[:], rstd_b[:])
            nc.scalar.activation(tmp[:], tmp[:], AF.Sqrt, bias=epst[:, 0:1])
            nc.vector.reciprocal(rstd_b[:], tmp[:])
            return negmu_b, rstd_b

        def normalize(tT, out, C, negmu_b, rstd_b, g, b):
            for ci in range(C):
                nc.vector.tensor_add(out[:, ci, :], tT[:, ci, :], negmu_b[:])
                nc.vector.tensor_mul(out[:, ci, :], out[:, ci, :], rstd_b[:])
                nc.vector.tensor_scalar(out[:, ci, :], out[:, ci, :],
                                        g[:, ci, :], b[:, ci, :],
                                        ALU.mult, ALU.add)

        def proj(pspool, out, W, aT, CE, CM, NA, evict):
            for mi in range(CM):
                for (o, s) in _nsplits(NA):
                    ps = pspool.tile([128, 512], f32, tag="proj_ps",
                                     name="proj_ps")
                    mm(ps[:, :s], [(W[:, ce, mi * 128:(mi + 1) * 128],
                                    aT[:, ce, o:o + s]) for ce in range(CE)])
                    evict(out[:, mi, o:o + s], ps[:, :s])

        # ---- P1: yT = (y0 @ W_res)^T -> DRAM scratch ----
        with tc.tile_pool(name="p1", bufs=1) as p1, \
             tc.tile_pool(name="p1e", bufs=3) as p1e, \
             tc.tile_pool(name="p1ps", bufs=2, space="PSUM") as p1ps:
            W_res = chunked(p1, "W_res")
            y0T = chunked(p1, "y0T")
            for mi in range(CD):
                ps = p1ps.tile([128, 512], f32, tag="proj_ps", name="proj_ps")
                mm(ps[:], [(W_res[:, ce, mi * 128:(mi + 1) * 128],
                            y0T[:, ce, :]) for ce in range(CY)])
                ysb = p1e.tile([128, N2], f32r, tag="ysb", name="ysb")
                ccopy(ysb[:], ps[:])
                nc.sync.dma_start(
                    bass.AP(ydram.tensor, mi * 128 * N2, [[N2, 128], [1, N2]]),
                    ysb[:])

        # ---- P2-P4: layernorms + projections (outputs live into P6) ----
        attp_cm = tc.tile_pool(name="attp", bufs=1)
        attp = attp_cm.__enter__()
        qT = attp.tile([128, CHD, NI], bf16, tag="qT")
        kT = attp.tile([128, CHD, N2], bf16, tag="kT")
        kbT = attp.tile([128, CHD, N2], bf16, tag="kbT")
        v2n = attp.tile([128, JT, H * DV], bf16, tag="v2n")
        v1n = attp.tile([128, IT, H * DV], bf16, tag="v1n")
        relqT = attp.tile([128, CHD, LREL], bf16, tag="relqT")

        # (a)+(b): y layernorm; kT, v2n
        with tc.tile_pool(name="py", bufs=1) as py, \
             tc.tile_pool(name="pyps", bufs=2, space="PSUM") as pyps:
            yT = py.tile([128, CD, N2], f32r, tag="yT")
            nc.sync.dma_start(yT[:],
                              ydram[:].rearrange("(c p n) -> p c n",
                                                 p=128, n=N2))
            negmu_y, rstd_y = stats_of(yT, CD, N2, pyps, py, "sy")
            y1T = py.tile([128, CD, N2], bf16, tag="y1T")
            normalize(yT, y1T, CD, negmu_y, rstd_y, vt["lny_g"], vt["lny_b"])
            Wk = chunked(py, "Wk")
            Wv2 = chunked(py, "Wv2")
            proj(pyps, kT, Wk, y1T, CD, CHD, N2, ccopy)
            proj(pyps, v2n, y1T, Wv2, CD, JT, H * DV, ccopy)
            for ci in range(CHD):
                nc.vector.tensor_scalar(kbT[:, ci, :], kT[:, ci, :],
                                        SCALE, vt["relb"][:, ci, :],
                                        ALU.mult, ALU.add)

        # rel_qT from pos
        with tc.tile_pool(name="pr", bufs=1) as pr, \
             tc.tile_pool(name="prps", bufs=2, space="PSUM") as prps:
            posT = pr.tile([NRP, LREL], bf16, tag="posT")
            nc.sync.dma_start(posT[:], d_in["posT"].ap())
            Wrel = pr.tile([NRP, H * DK], bf16, tag="Wrel")
            nc.sync.dma_start(Wrel[:], d_in["Wrel"].ap())
            for mi in range(CHD):
                for (o, s) in _nsplits(LREL):
                    ps = prps.tile([128, 512], f32, tag="proj_ps",
                                   name="proj_ps")
                    mm(ps[:, :s], [(Wrel[:, mi * 128:(mi + 1) * 128],
                                    posT[:, o:o + s])])
                    ccopy(relqT[:, mi, o:o + s], ps[:, :s])

        # (c)+(d): x layernorm (per i-half); qT, v1n
        with tc.tile_pool(name="px", bufs=1) as px, \
             tc.tile_pool(name="pxps", bufs=2, space="PSUM") as pxps:
            xTt = chunked(px, "xT")
            x1T = px.tile([128, CD, NI], bf16, tag="x1T")
            for half in range(2):
                o = half * 512
                xh = xTt[:, :, o:o + 512]
                negmu_x, rstd_x = stats_of(xh, CD, 512, pxps, px, "sx")
                normalize(xh, x1T[:, :, o:o + 512], CD, negmu_x, rstd_x,
                          vt["lnx_g"], vt["lnx_b"])
            Wq = chunked(px, "Wq")
            Wv1 = chunked(px, "Wv1")
            proj(pxps, qT, Wq, x1T, CD, CHD, NI,
                 lambda o, p: nc.scalar.mul(o, p, SCALE))
            proj(pxps, v1n, x1T, Wv1, CD, IT, H * DV, ccopy)

        # ---- P5: rel[j, m] per head -> DRAM (row pitch LREL) ----
        relbufs = [dram.tile([N2 * LREL], bf16, tag=f"relbuf{h}",
                             name=f"relbuf{h}")
                   for h in range(H)]
        with tc.tile_pool(name="p5", bufs=3) as p5, \
             tc.tile_pool(name="p5ps", bufs=2, space="PSUM") as p5ps:
            for h in range(H):
                hc, ho = h // 2, (h % 2) * DK
                for jt in range(JT):
                    ps = p5ps.tile([128, LREL], f32, tag="rel_ps",
                                   name="rel_ps")
                    kb_l = kbT[:, hc, jt * 128:(jt + 1) * 128][ho:ho + DK, :]
                    for (o, s) in _nsplits(LREL):
                        nc.tensor.matmul(
                            ps[:, o:o + s], kb_l,
                            relqT[:, hc, o:o + s][ho:ho + DK, :],
                            start=True, stop=True)
                    sb = p5.tile([128, LREL], bf16, tag="rel_sb", name="rel_sb")
                    ccopy(sb[:], ps[:])
                    nc.sync.dma_start(
                        bass.AP(relbufs[h].tensor, jt * 128 * LREL,
                                [[LREL, 128], [1, LREL]]),
                        sb[:])

        # ---- P6: attention head loop ----
        ar_in = dram.tile([H * DV, N2], f32, name="ar_in")
        ar_out = dram.tile([H * DV, N2], f32, name="ar_out")

        with tc.tile_pool(name="p6relw", bufs=3) as p6relw, \
             tc.tile_pool(name="p6et", bufs=5) as p6et, \
             tc.tile_pool(name="p6e", bufs=3) as p6e, \
             tc.tile_pool(name="p6sm", bufs=2) as p6sm, \
             tc.tile_pool(name="psct", bufs=2, space="PSUM") as psct, \
             tc.tile_pool(name="psrs", bufs=1, space="PSUM") as psrs, \
             tc.tile_pool(name="pso1", bufs=1, space="PSUM") as pso1, \
             tc.tile_pool(name="pso2", bufs=1, space="PSUM") as pso2:
            for h in range(H):
                hc, ho = h // 2, (h % 2) * DK
                # E^T = exp(content^T + rel_shift) in [j, i] layout
                ets = []
                for jt in range(JT):
                    relw = p6relw.tile([128, NI], bf16, tag="relw",
                                       name="relw")
                    nc.sync.dma_start(
                        relw[:],
                        bass.AP(relbufs[h].tensor,
                                jt * 128 * (LREL - 1) + (N2 - 1),
                                [[LREL - 1, 128], [1, NI]]))
                    et = p6et.tile([128, NI], bf16, tag="et", name="et")
                    k_l = kT[:, hc, jt * 128:(jt + 1) * 128][ho:ho + DK, :]
                    ps = psct.tile([128, NI], f32, tag="ct", name="ct")
                    for (o, s) in _nsplits(NI):
                        nc.tensor.matmul(ps[:, o:o + s], k_l,
                                         qT[:, hc, o:o + s][ho:ho + DK, :],
                                         start=True, stop=True)
                    nc.vector.tensor_tensor(ps[:], ps[:], relw[:], ALU.add)
                    nc.scalar.activation(et[:], ps[:], AF.Exp)
                    ets.append(et)
                # out1^T (unnormalized) [DV, NI]
                ps_o1 = pso1.tile([DV, NI], f32, tag="o1", name="o1")
                for (o, s) in _nsplits(NI):
                    for jt in range(JT):
                        nc.tensor.matmul(
                            ps_o1[:, o:o + s],
                            v2n[:, jt, h * DV:(h + 1) * DV],
                            ets[jt][:, o:o + s],
                            start=(jt == 0), stop=(jt == JT - 1))
                # row-sums 1/rs: ones-matmul over ET (j on partitions)
                rsrow = p6sm.tile([1, NI], f32, tag="rsrow", name="rsrow")
                for (o, s) in _nsplits(NI):
                    ps_r = psrs.tile([1, 512], f32, tag="rs", name="rs")
                    for jt in range(JT):
                        nc.tensor.matmul(ps_r[:, :s], ones_bf[:, 0:1],
                                         ets[jt][:, o:o + s],
                                         start=(jt == 0), stop=(jt == JT - 1))
                    ccopy(rsrow[:, o:o + s], ps_r[:, :s])
                rsrecip = p6sm.tile([1, NI], f32, tag="rsrecip",
                                    name="rsrecip")
                nc.vector.reciprocal(rsrecip[:], rsrow[:])
                rsdram = dram.tile([NI], f32, tag=f"rsdram{h}",
                                   name=f"rsdram{h}")
                nc.sync.dma_start(
                    bass.AP(rsdram.tensor, 0, [[NI, 1], [1, NI]]), rsrecip[:])
                # E blocks [i, j] via DMA transpose (bf16)
                es = []
                for it in range(IT):
                    e = p6e.tile([128, N2], bf16, tag="e", name="e")
                    for jt in range(JT):
                        nc.sync.dma_start_transpose(
                            e[:, jt * 128:(jt + 1) * 128],
                            ets[jt][:, it * 128:(it + 1) * 128])
                    es.append(e)
                rsrow_b = p6sm.tile([DV, NI], f32, tag="rsrow_b",
                                    name="rsrow_b")
                nc.gpsimd.dma_start(
                    out=rsrow_b[:],
                    in_=bass.AP(rsdram.tensor, 0, [[0, DV], [1, NI]]))
                # out1 normalized -> DRAM rows [h*96, +96]
                o1h = p6sm.tile([DV, NI], bf16, tag="o1h", name="o1h")
                nc.vector.tensor_mul(o1h[:], ps_o1[:], rsrow_b[:])
                nc.sync.dma_start(
                    bass.AP(o1dram.tensor, h * DV * NI, [[NI, DV], [1, NI]]),
                    o1h[:])
                # out2^T partial [DV, N2]: v1 rows scaled by 1/rs, then
                # contract over i
                ps_o2 = pso2.tile([DV, N2], f32, tag="o2", name="o2")
                for it in range(IT):
                    rsc = p6sm.tile([128, 1], f32, tag="rsc", name="rsc")
                    nc.sync.dma_start(
                        rsc[:],
                        bass.AP(rsdram.tensor, it * 128, [[1, 128], [1, 1]]))
                    v1p = p6sm.tile([128, DV], bf16, tag="v1p", name="v1p")
                    nc.vector.tensor_scalar(
                        v1p[:], v1n[:, it, h * DV:(h + 1) * DV],
                        rsc[:, 0:1], None, ALU.mult)
                    nc.tensor.matmul(ps_o2[:], v1p[:], es[it][:],
                                     start=(it == 0), stop=(it == IT - 1))
                o2h = p6sm.tile([DV, N2], f32, tag="o2h", name="o2h")
                ccopy(o2h[:], ps_o2[:])
                nc.sync.dma_start(ar_in[h * DV:(h + 1) * DV, :], o2h[:])
        attp_cm.__exit__(None, None, None)

        # ---- P7: pair AllReduce of out2 partials ----
        nc.gpsimd.collective_compute(
            "AllReduce", ALU.add,
            replica_groups=[[0, 1], [2, 3], [4, 5], [6, 7]],
            ins=[ar_in[:].opt()], outs=[ar_out[:].opt()])

        # ---- P8: x2 + residual + FFN-x, per i-half ----
        with tc.tile_pool(name="p8w", bufs=1) as p8w, \
             tc.tile_pool(name="p8", bufs=1) as p8, \
             tc.tile_pool(name="p8h", bufs=2) as p8h, \
             tc.tile_pool(name="p8s", bufs=3) as p8s, \
             tc.tile_pool(name="p8w1", bufs=3) as p8w1, \
             tc.tile_pool(name="p8w2", bufs=4) as p8w2, \
             tc.tile_pool(name="p8ps", bufs=2, space="PSUM") as p8ps, \
             tc.tile_pool(name="p8psb", bufs=2, space="PSUM") as p8psb:
            Wo1 = chunked(p8w, "Wo1")
            for half in range(2):
                o = half * 512
                xTh = p8h.tile([128, CD, 512], f32r, tag="xTh", name="xTh")
                nc.sync.dma_start(
                    xTh[:], d_in["xT"].ap()[:, o:o + 512]
                    .rearrange("(c p) n -> p c n", p=128))
                o1c = p8h.tile([128, CD, 512], bf16, tag="o1c", name="o1c")
                nc.sync.dma_start(
                    o1c[:],
                    o1dram[:].rearrange("(c p n) -> p c n",
                                        p=128, n=NI)[:, :, o:o + 512])
                x4T = p8.tile([128, CD, 512], f32r, tag="x4T", name="x4T")
                for mi in range(CD):
                    ps = p8ps.tile([128, 512], f32, tag="x2ps", name="x2ps")
                    mm(ps[:], [(Wo1[:, ce, mi * 128:(mi + 1) * 128],
                                o1c[:, ce, :]) for ce in range(CD)])
                    nc.vector.scalar_tensor_tensor(
                        x4T[:, mi, :], ps[:], vt["bo1"][:, mi, :],
                        xTh[:, mi, :], ALU.add, ALU.add)
                negmu, rstd = stats_of(x4T, CD, 512, p8ps, p8, "s4")
                x4ln = p8.tile([128, CD, 512], bf16, tag="x4ln", name="x4ln")
                normalize(x4T, x4ln, CD, negmu, rstd, vt["fx_g"], vt["fx_b"])
                h1 = p8.tile([128, CF, 512], bf16, tag="h1", name="h1")
                for fc in range(CF):
                    w1c = p8w1.tile([128, CD, 128], bf16, tag="w1c",
                                    name="w1c")
                    nc.sync.dma_start(
                        w1c[:], d_in["fx_w1"].ap()[:, fc * 128:(fc + 1) * 128]
                        .rearrange("(c p) n -> p c n", p=128))
                    ps_h = p8psb.tile([128, 512], f32, tag="hps", name="hps")
                    mm(ps_h[:], [(w1c[:, ce, :], x4ln[:, ce, :])
                                 for ce in range(CD)])
                    nc.scalar.activation(h1[:, fc, :], ps_h[:], AF.Relu,
                                         bias=vt["fx_b1"][:, fc, :])
                for mi in range(CD):
                    ps = p8psb.tile([128, 512], f32, tag="x5ps", name="x5ps")
                    for fc in range(CF):
                        w2b = p8w2.tile([128, 128], bf16, tag="w2b",
                                        name="w2b")
                        nc.sync.dma_start(
                            w2b[:],
                            d_in["fx_w2"].ap()[fc * 128:(fc + 1) * 128,
                                               mi * 128:(mi + 1) * 128])
                        nc.tensor.matmul(ps[:], w2b[:], h1[:, fc, :],
                                         start=(fc == 0), stop=(fc == CF - 1))
                    x5s = p8s.tile([128, 512], f32, tag="x5s", name="x5s")
                    nc.vector.scalar_tensor_tensor(
                        x5s[:], ps[:], vt["fx_b2"][:, mi, :],
                        x4T[:, mi, :], ALU.add, ALU.add)
                    nc.sync.dma_start(
                        x5T_d.ap()[mi * 128:(mi + 1) * 128, o:o + 512],
                        x5s[:])

        # ---- P9: y2 + residual + FFN-y (full width, duplicated in pair) ----
        with tc.tile_pool(name="p9w", bufs=1) as p9w, \
             tc.tile_pool(name="p9", bufs=1) as p9, \
             tc.tile_pool(name="p9s", bufs=3) as p9s, \
             tc.tile_pool(name="p9w2", bufs=4) as p9w2, \
             tc.tile_pool(name="p9ps", bufs=2, space="PSUM") as p9ps, \
             tc.tile_pool(name="p9psb", bufs=2, space="PSUM") as p9psb:
            Wo2 = chunked(p9w, "Wo2")
            fy_w1 = chunked(p9w, "fy_w1")
            o2r = p9.tile([128, CD, N2], bf16, tag="o2r", name="o2r")
            nc.gpsimd.dma_start(
                out=o2r[:], in_=ar_out[:].rearrange("(c p) n -> p c n", p=128))
            yTr = p9.tile([128, CD, N2], f32r, tag="yTr", name="yTr")
            nc.sync.dma_start(yTr[:],
                              ydram[:].rearrange("(c p n) -> p c n",
                                                 p=128, n=N2))
            y4T = p9.tile([128, CD, N2], f32r, tag="y4T", name="y4T")
            for mi in range(CD):
                ps = p9ps.tile([128, 512], f32, tag="y2ps", name="y2ps")
                mm(ps[:], [(Wo2[:, ce, mi * 128:(mi + 1) * 128],
                            o2r[:, ce, :]) for ce in range(CD)])
                nc.vector.scalar_tensor_tensor(
                    y4T[:, mi, :], ps[:], vt["bo2"][:, mi, :],
                    yTr[:, mi, :], ALU.add, ALU.add)
            negmu, rstd = stats_of(y4T, CD, N2, p9ps, p9, "s5")
            y4ln = p9.tile([128, CD, N2], bf16, tag="y4ln", name="y4ln")
            normalize(y4T, y4ln, CD, negmu, rstd, vt["fy_g"], vt["fy_b"])
            h1y = p9.tile([128, CF, N2], bf16, tag="h1y", name="h1y")
            for fc in range(CF):
                ps_h = p9psb.tile([128, 512], f32, tag="hyps", name="hyps")
                mm(ps_h[:], [(fy_w1[:, ce, fc * 128:(fc + 1) * 128],
                              y4ln[:, ce, :]) for ce in range(CD)])
                nc.scalar.activation(h1y[:, fc, :], ps_h[:], AF.Relu,
                                     bias=vt["fy_b1"][:, fc, :])
            for mi in range(CD):
                ps = p9psb.tile([128, 512], f32, tag="y5ps", name="y5ps")
                for fc in range(CF):
                    w2b = p9w2.tile([128, 128], bf16, tag="yw2b", name="yw2b")
                    nc.sync.dma_start(
                        w2b[:],
                        d_in["fy_w2"].ap()[fc * 128:(fc + 1) * 128,
                                           mi * 128:(mi + 1) * 128])
                    nc.tensor.matmul(ps[:], w2b[:], h1y[:, fc, :],
                                     start=(fc == 0), stop=(fc == CF - 1))
                y5s = p9s.tile([128, 512], f32, tag="y5s", name="y5s")
                nc.vector.scalar_tensor_tensor(
                    y5s[:], ps[:], vt["fy_b2"][:, mi, :],
                    y4T[:, mi, :], ALU.add, ALU.add)
                nc.sync.dma_start(y5T_d.ap()[mi * 128:(mi + 1) * 128, :],
                                  y5s[:])


_CACHE = {}
LAST_RESULT = None


def kernel(**inputs):
    if "nc" not in _CACHE:
        _CACHE["nc"] = build_bass()
        _CACHE["pos"] = _positional_embed()
    nc = _CACHE["nc"]
    pos = _CACHE["pos"]

    f = {k: np.ascontiguousarray(np.asarray(v, dtype=np.float32))
         for k, v in inputs.items()}
    col = lambda a: np.ascontiguousarray(a.reshape(-1, 1))
    bf = lambda a: np.ascontiguousarray(a.astype(ml_dtypes.bfloat16))
    shared = dict(
        W_res=f["W_res"], Wq=bf(f["Wq"]), Wk=bf(f["Wk"]), Wv1=bf(f["Wv1"]),
        Wv2=bf(f["Wv2"]),
        Wo1=bf(f["Wo1"]), Wo2=bf(f["Wo2"]), Wrel=bf(f["Wrel"]),
        fx_w1=bf(f["fx_w1"]), fx_w2=bf(f["fx_w2"]), fy_w1=bf(f["fy_w1"]),
        fy_w2=bf(f["fy_w2"]),
        lnx_g=col(f["lnx_g"]), lnx_b=col(f["lnx_b"]),
        lny_g=col(f["lny_g"]), lny_b=col(f["lny_b"]),
        fx_g=col(f["fx_g"]), fx_b=col(f["fx_b"]),
        fy_g=col(f["fy_g"]), fy_b=col(f["fy_b"]),
        bo1=col(f["bo1"]), bo2=col(f["bo2"]),
        fx_b2=col(f["fx_b2"]), fy_b2=col(f["fy_b2"]),
        fx_b1=col(f["fx_b1"]), fy_b1=col(f["fy_b1"]),
        relb=col(f["rel_pos_bias"]),
    )
    in_maps = []
    for c in range(NCORES):
        b, ih = divmod(c, 2)
        i0 = ih * NI
        m = dict(shared)
        m["xT"] = np.ascontiguousarray(f["x"][b, i0:i0 + NI, :].T)
        m["y0T"] = np.ascontiguousarray(f["y0"][b].T)
        m["posT"] = bf(pos[i0:i0 + LREL].T)
        in_maps.append(m)

    import os
    kwargs = {}
    if os.environ.get("KERNEL_TRACE"):
        kwargs = dict(trace=True,
                      trace_cores=[int(v) for v in
                                   os.environ.get("KERNEL_TRACE_CORES",
                                                  "0").split(",")])
    res = bass_utils.run_bass_kernel_spmd(nc, in_maps,
                                          core_ids=list(range(NCORES)),
                                          **kwargs)
    global LAST_RESULT
    LAST_RESULT = res
    x5 = np.empty((B, N1, D), np.float32)
    y5 = np.empty((B, N2, D), np.float32)
    for c in range(NCORES):
        b, ih = divmod(c, 2)
        x5[b, ih * NI:(ih + 1) * NI, :] = res.results[c]["x5T"].T
        if ih == 0:
            y5[b] = res.results[c]["y5T"].T
    return x5, y5


# revision 25
# speedup vs baseline: 1.5309x; 1.5309x over previous
"""CrossFormer layer (nn_CrossFormerLayer) on 8 trn2 NeuronCores.

Sharding: core c -> batch b = c//2, i-half ih = c%2 (1024 of the 2048
n1 rows). The x-path is fully local to a core; the y-path needs one
pair-wise (2-core) AllReduce of the partial out2 = attn^T @ v1 (the
contraction over i is split across the pair). The y tail
(out2 @ Wo2 + FFN) is duplicated inside each pair (cheap).

All activations are kept in transposed [feature, seq] layout so every
matmul chains without activation transposes. The attention matrix E
(= exp(logits), softmax numerator) is the one tensor needed in both
orientations; it is PE-transposed per 128x128 block. Softmax skips the
max-subtraction (logits are O(1) by construction). The Transformer-XL
relative shift is free: rel is stored [j, m] with row pitch 1535 in
DRAM and the shifted matrix is a strided read
shifted[j, i] = flat[j*1534 + 511 + i].
"""
import math
import ml_dtypes
import numpy as np

import concourse.bacc as bacc
import concourse.bass as bass
import concourse.mybir as mybir
import concourse.tile as tile
from concourse import bass_utils
from concourse.masks import make_identity

f32 = mybir.dt.float32
f32r = mybir.dt.float32r
bf16 = mybir.dt.bfloat16
AF = mybir.ActivationFunctionType
ALU = mybir.AluOpType

B, N1, N2 = 4, 2048, 512
D, H, DK, DV, NRP = 768, 8, 64, 96, 96
DY0 = 1536
DFF = 2 * D
NI = N1 // 2              # 1024 i-rows per core
LREL = N2 + NI            # 1536 rel columns (local m window, padded even)
EPS = 1e-5
SCALE = DK ** -0.5
NCORES = 8
CD = D // 128             # 6
CY = DY0 // 128           # 12
CHD = (H * DK) // 128     # 4
CF = DFF // 128           # 12
JT = N2 // 128            # 4
IT = NI // 128            # 8


def _positional_embed():
    """Enformer relative positional features, pure numpy (fp64->fp32)."""
    n1, fs = N1, NRP
    d = np.arange(-n1 + 1, n1)
    ad = np.abs(d).astype(np.float64)[:, None]
    nb = fs // 6
    max_range = math.log(n1) / math.log(2.0)
    half_life = 2.0 ** np.linspace(3.0, max_range, nb)
    f_exp = np.exp(-math.log(2.0) / half_life * ad)
    cw = 2.0 ** np.arange(1, nb + 1).astype(np.float64) - 1.0
    f_cm = (cw > ad).astype(np.float64)
    stddev = n1 / (2.0 * nb)
    start_mean = n1 / nb
    mean = np.linspace(start_mean, float(n1), nb)
    conc = (mean / stddev) ** 2
    rate = mean / (stddev ** 2)
    lgamma = np.array([math.lgamma(c) for c in conc])
    ad_safe = np.where(ad > 0, ad, 1.0)
    logp = (conc - 1.0) * np.log(ad_safe) - rate * ad - (lgamma - conc * np.log(rate))
    logp = np.where(ad > 0, logp, -np.inf)
    prob = np.exp(logp) + 1e-8
    f_g = prob / np.max(prob, -1, keepdims=True)
    emb = np.concatenate([f_exp, f_cm, f_g], -1)
    sign = np.sign(d).astype(np.float64)[:, None]
    return np.concatenate([emb, sign * emb], -1).astype(np.float32)  # [4095, 96]


def _nsplits(n):
    out, o = [], 0
    while o < n:
        s = min(512, n - o)
        out.append((o, s))
        o += s
    return out


def build_bass():
    nc = bacc.Bacc("TRN2", target_bir_lowering=False, debug=False,
                   num_devices=NCORES)

    F32R_INPUTS = {"xT", "y0T", "W_res"}
    BF16_INPUTS = {"posT", "Wq", "Wk", "Wv1", "Wv2", "Wo1", "Wo2", "Wrel",
                   "fx_w1", "fx_w2", "fy_w1", "fy_w2"}

    def din(name, shape):
        dt = (f32r if name in F32R_INPUTS
              else bf16 if name in BF16_INPUTS else f32)
        return nc.dram_tensor(name, shape, dt, kind="ExternalInput")

    d_in = {}
    for nm, shape in [
        ("xT", [D, NI]), ("y0T", [DY0, N2]), ("posT", [NRP, LREL]),
        ("W_res", [DY0, D]), ("Wq", [D, H * DK]), ("Wk", [D, H * DK]),
        ("Wv1", [D, H * DV]), ("Wv2", [D, H * DV]),
        ("Wo1", [H * DV, D]), ("Wo2", [H * DV, D]), ("Wrel", [NRP, H * DK]),
        ("fx_w1", [D, DFF]), ("fx_w2", [DFF, D]),
        ("fy_w1", [D, DFF]), ("fy_w2", [DFF, D]),
        ("lnx_g", [D, 1]), ("lnx_b", [D, 1]), ("lny_g", [D, 1]),
        ("lny_b", [D, 1]), ("fx_g", [D, 1]), ("fx_b", [D, 1]),
        ("fy_g", [D, 1]), ("fy_b", [D, 1]), ("bo1", [D, 1]), ("bo2", [D, 1]),
        ("fx_b2", [D, 1]), ("fy_b2", [D, 1]), ("fx_b1", [DFF, 1]),
        ("fy_b1", [DFF, 1]), ("relb", [H * DK, 1]),
    ]:
        d_in[nm] = din(nm, shape)
    x5T_d = nc.dram_tensor("x5T", [D, NI], f32, kind="ExternalOutput")
    y5T_d = nc.dram_tensor("y5T", [D, N2], f32, kind="ExternalOutput")

    with tile.TileContext(nc) as tc:
        _build(nc, tc, d_in, x5T_d, y5T_d)
    nc.compile()
    return nc


def _build(nc, tc, d_in, x5T_d, y5T_d):
    def mm(ps_ap, pairs):
        n = len(pairs)
        for i, (l, r) in enumerate(pairs):
            nc.tensor.matmul(ps_ap, l, r, start=(i == 0), stop=(i == n - 1))

    def chunked(pool, name, tag=None):
        d = d_in[name]
        rows, cols = d.shape
        c = (rows + 127) // 128
        t = pool.tile([128, c, cols], d.dtype, tag=(tag or name),
                      name=(tag or name))
        nc.sync.dma_start(t[:], d.ap().rearrange("(c p) n -> p c n", p=128))
        return t

    ccopy = nc.scalar.copy

    with tc.tile_pool(name="const", bufs=1) as cpool, \
         tc.tile_pool(name="dram", bufs=1, space="DRAM") as dram:

        stage32 = cpool.tile([128, 128], f32)
        ones128 = cpool.tile([128, 128], f32r)
        nc.vector.memset(stage32[:], 1.0)
        ccopy(ones128[:], stage32[:])
        ones_bf = cpool.tile([128, 1], bf16)
        ccopy(ones_bf[:], stage32[:, 0:1])
        ident32 = cpool.tile([128, 128], f32)
        make_identity(nc, ident32[:])
        ident = cpool.tile([128, 128], bf16)
        ccopy(ident[:], ident32[:])
        epst = cpool.tile([128, 1], f32)
        nc.vector.memset(epst[:], EPS)
        vt = {nm: chunked(cpool, nm) for nm in
              ["lnx_g", "lnx_b", "lny_g", "lny_b", "fx_g", "fx_b", "fy_g",
               "fy_b", "bo1", "bo2", "fx_b2", "fy_b2", "fx_b1", "fy_b1",
               "relb"]}

        ydram = dram.tile([D * N2], f32r, name="ydram")
        o1dram = dram.tile([H * DV * NI], bf16, name="o1dram")

        def stats_of(tT, C, N, pspool, spool, tag):
            """LN stats over the feature dim (C*128). The all-ones [128,128]
            lhsT replicates column sums to every partition, so the stats come
            out already broadcast: returns (negmu_b, rstd_b) [128, N]."""
            nfeat = C * 128
            negmu_b = spool.tile([128, N], f32, tag=tag + "_mub",
                                 name=tag + "_mub")
            rstd_b = spool.tile([128, N], f32, tag=tag + "_rsb",
                                name=tag + "_rsb")
            tmp = spool.tile([128, N], f32, tag=tag + "_tmp",
                             name=tag + "_tmp")
            for (o, s) in _nsplits(N):
                ps_s = pspool.tile([128, 512], f32, tag="stat_ps",
                                   name="stat_ps")
                mm(ps_s[:, :s],
                   [(ones128[:], tT[:, ci, o:o + s]) for ci in range(C)])
                nc.scalar.mul(negmu_b[:, o:o + s], ps_s[:, :s], -1.0 / nfeat)
                ps_q = pspool.tile([128, 512], f32, tag="stat_ps",
                                   name="stat_ps")
                for ci in range(C):
                    sq = spool.tile([128, 512], f32r, tag=tag + "_sqb",
                                    name=tag + "_sqb")
                    nc.scalar.square(sq[:, :s], tT[:, ci, o:o + s])
                    nc.tensor.matmul(ps_q[:, :s], ones128[:], sq[:, :s],
                                     start=(ci == 0), stop=(ci == C - 1))
                nc.scalar.mul(tmp[:, o:o + s], ps_q[:, :s], 1.0 / nfeat)
            nc.vector.tensor_mul(rstd_b[:], negmu_b[:], negmu_b[:])
            nc.vector.tensor_sub(tmp[:], tmp[:], rstd_b[:])
            nc.scalar.activation(tmp[:], tmp[:], AF.Sqrt, bias=epst[:, 0:1])
            nc.vector.reciprocal(rstd_b[:], tmp[:])
            return negmu_b, rstd_b

        def normalize(tT, out, C, negmu_b, rstd_b, g, b):
            for ci in range(C):
                nc.vector.tensor_add(out[:, ci, :], tT[:, ci, :], negmu_b[:])
                nc.vector.tensor_mul(out[:, ci, :], out[:, ci, :], rstd_b[:])
                nc.vector.tensor_scalar(out[:, ci, :], out[:, ci, :],
                                        g[:, ci, :], b[:, ci, :],
                                        ALU.mult, ALU.add)

        def proj(pspool, out, W, aT, CE, CM, NA, evict):
            for mi in range(CM):
                for (o, s) in _nsplits(NA):
                    ps = pspool.tile([128, 512], f32, tag="proj_ps",
                                     name="proj_ps")
                    mm(ps[:, :s], [(W[:, ce, mi * 128:(mi + 1) * 128],
                                    aT[:, ce, o:o + s]) for ce in range(CE)])
                    evict(out[:, mi, o:o + s], ps[:, :s])

        # ---- P1: yT = (y0 @ W_res)^T -> DRAM scratch ----
        with tc.tile_pool(name="p1", bufs=1) as p1, \
             tc.tile_pool(name="p1e", bufs=3) as p1e, \
             tc.tile_pool(name="p1ps", bufs=2, space="PSUM") as p1ps:
            W_res = chunked(p1, "W_res")
            y0T = chunked(p1, "y0T")
            for mi in range(CD):
                ps = p1ps.tile([128, 512], f32, tag="proj_ps", name="proj_ps")
                mm(ps[:], [(W_res[:, ce, mi * 128:(mi + 1) * 128],
                            y0T[:, ce, :]) for ce in range(CY)])
                ysb = p1e.tile([128, N2], f32r, tag="ysb", name="ysb")
                ccopy(ysb[:], ps[:])
                nc.sync.dma_start(
                    bass.AP(ydram.tensor, mi * 128 * N2, [[N2, 128], [1, N2]]),
                    ysb[:])

        # ---- P2-P4: layernorms + projections (outputs live into P6) ----
        attp_cm = tc.tile_pool(name="attp", bufs=1)
        attp = attp_cm.__enter__()
        qT = attp.tile([128, CHD, NI], bf16, tag="qT")
        kT = attp.tile([128, CHD, N2], bf16, tag="kT")
        kbT = attp.tile([128, CHD, N2], bf16, tag="kbT")
        v2n = attp.tile([128, JT, H * DV], bf16, tag="v2n")
        v1n = attp.tile([128, IT, H * DV], bf16, tag="v1n")
        relqT = attp.tile([128, CHD, LREL], bf16, tag="relqT")

        # (a)+(b): y layernorm; kT, v2n
        with tc.tile_pool(name="py", bufs=1) as py, \
             tc.tile_pool(name="pyps", bufs=2, space="PSUM") as pyps:
            yT = py.tile([128, CD, N2], f32r, tag="yT")
            nc.sync.dma_start(yT[:],
                              ydram[:].rearrange("(c p n) -> p c n",
                                                 p=128, n=N2))
            negmu_y, rstd_y = stats_of(yT, CD, N2, pyps, py, "sy")
            y1T = py.tile([128, CD, N2], bf16, tag="y1T")
            normalize(yT, y1T, CD, negmu_y, rstd_y, vt["lny_g"], vt["lny_b"])
            Wk = chunked(py, "Wk")
            Wv2 = chunked(py, "Wv2")
            proj(pyps, kT, Wk, y1T, CD, CHD, N2, ccopy)
            proj(pyps, v2n, y1T, Wv2, CD, JT, H * DV, ccopy)
            for ci in range(CHD):
                nc.vector.tensor_scalar(kbT[:, ci, :], kT[:, ci, :],
                                        SCALE, vt["relb"][:, ci, :],
                                        ALU.mult, ALU.add)

        # rel_qT from pos
        with tc.tile_pool(name="pr", bufs=1) as pr, \
             tc.tile_pool(name="prps", bufs=2, space="PSUM") as prps:
            posT = pr.tile([NRP, LREL], bf16, tag="posT")
            nc.sync.dma_start(posT[:], d_in["posT"].ap())
            Wrel = pr.tile([NRP, H * DK], bf16, tag="Wrel")
            nc.sync.dma_start(Wrel[:], d_in["Wrel"].ap())
            for mi in range(CHD):
                for (o, s) in _nsplits(LREL):
                    ps = prps.tile([128, 512], f32, tag="proj_ps",
                                   name="proj_ps")
                    mm(ps[:, :s], [(Wrel[:, mi * 128:(mi + 1) * 128],
                                    posT[:, o:o + s])])
                    ccopy(relqT[:, mi, o:o + s], ps[:, :s])

        # (c)+(d): x layernorm (per i-half); qT, v1n
        with tc.tile_pool(name="px", bufs=1) as px, \
             tc.tile_pool(name="pxps", bufs=2, space="PSUM") as pxps:
            xTt = chunked(px, "xT")
            x1T = px.tile([128, CD, NI], bf16, tag="x1T")
            for half in range(2):
                o = half * 512
                xh = xTt[:, :, o:o + 512]
                negmu_x, rstd_x = stats_of(xh, CD, 512, pxps, px, "sx")
                normalize(xh, x1T[:, :, o:o + 512], CD, negmu_x, rstd_x,
                          vt["lnx_g"], vt["lnx_b"])
            Wq = chunked(px, "Wq")
            Wv1 = chunked(px, "Wv1")
            proj(pxps, qT, Wq, x1T, CD, CHD, NI,
                 lambda o, p: nc.scalar.mul(o, p, SCALE))
            proj(pxps, v1n, x1T, Wv1, CD, IT, H * DV, ccopy)

        # ---- P5: rel[j, m] per head -> DRAM (row pitch LREL) ----
        relbufs = [dram.tile([N2 * LREL], bf16, tag=f"relbuf{h}",
                             name=f"relbuf{h}")
                   for h in range(H)]
        with tc.tile_pool(name="p5", bufs=3) as p5, \
             tc.tile_pool(name="p5ps", bufs=2, space="PSUM") as p5ps:
            for h in range(H):
                hc, ho = h // 2, (h % 2) * DK
                for jt in range(JT):
                    ps = p5ps.tile([128, LREL], f32, tag="rel_ps",
                                   name="rel_ps")
                    kb_l = kbT[:, hc, jt * 128:(jt + 1) * 128][ho:ho + DK, :]
                    for (o, s) in _nsplits(LREL):
                        nc.tensor.matmul(
                            ps[:, o:o + s], kb_l,
                            relqT[:, hc, o:o + s][ho:ho + DK, :],
                            start=True, stop=True)
                    sb = p5.tile([128, LREL], bf16, tag="rel_sb", name="rel_sb")
                    ccopy(sb[:], ps[:])
                    nc.sync.dma_start(
                        bass.AP(relbufs[h].tensor, jt * 128 * LREL,
                                [[LREL, 128], [1, LREL]]),
                        sb[:])

        # ---- P6: attention head loop ----
        ar_in = dram.tile([H * DV, N2], f32, name="ar_in")
        ar_out = dram.tile([H * DV, N2], f32, name="ar_out")

        with tc.tile_pool(name="p6relw", bufs=3) as p6relw, \
             tc.tile_pool(name="p6et", bufs=5) as p6et, \
             tc.tile_pool(name="p6e", bufs=3) as p6e, \
             tc.tile_pool(name="p6sm", bufs=2) as p6sm, \
             tc.tile_pool(name="psct", bufs=2, space="PSUM") as psct, \
             tc.tile_pool(name="pstr", bufs=2, space="PSUM") as pstr, \
             tc.tile_pool(name="psrs", bufs=1, space="PSUM") as psrs, \
             tc.tile_pool(name="pso1", bufs=1, space="PSUM") as pso1, \
             tc.tile_pool(name="pso2", bufs=1, space="PSUM") as pso2:
            for h in range(H):
                hc, ho = h // 2, (h % 2) * DK
                # E^T = exp(content^T + rel_shift) in [j, i] layout
                ets = []
                for jt in range(JT):
                    relw = p6relw.tile([128, NI], bf16, tag="relw",
                                       name="relw")
                    nc.sync.dma_start(
                        relw[:],
                        bass.AP(relbufs[h].tensor,
                                jt * 128 * (LREL - 1) + (N2 - 1),
                                [[LREL - 1, 128], [1, NI]]))
                    et = p6et.tile([128, NI], bf16, tag="et", name="et")
                    k_l = kT[:, hc, jt * 128:(jt + 1) * 128][ho:ho + DK, :]
                    for (o, s) in _nsplits(NI):
                        ps = psct.tile([128, 512], f32, tag="ct", name="ct")
                        nc.tensor.matmul(ps[:, :s], k_l,
                                         qT[:, hc, o:o + s][ho:ho + DK, :],
                                         start=True, stop=True)
                        nc.vector.tensor_tensor(ps[:, :s], ps[:, :s],
                                                relw[:, o:o + s], ALU.add)
                        nc.scalar.activation(et[:, o:o + s], ps[:, :s],
                                             AF.Exp)
                    ets.append(et)
                # out1^T (unnormalized) [DV, NI]
                ps_o1 = pso1.tile([DV, NI], f32, tag="o1", name="o1")
                for (o, s) in _nsplits(NI):
                    for jt in range(JT):
                        nc.tensor.matmul(
                            ps_o1[:, o:o + s],
                            v2n[:, jt, h * DV:(h + 1) * DV],
                            ets[jt][:, o:o + s],
                            start=(jt == 0), stop=(jt == JT - 1))
                # row-sums 1/rs: ones-matmul over ET (j on partitions)
                rsrow = p6sm.tile([1, NI], f32, tag="rsrow", name="rsrow")
                for (o, s) in _nsplits(NI):
                    ps_r = psrs.tile([1, 512], f32, tag="rs", name="rs")
                    for jt in range(JT):
                        nc.tensor.matmul(ps_r[:, :s], ones_bf[:, 0:1],
                                         ets[jt][:, o:o + s],
                                         start=(jt == 0), stop=(jt == JT - 1))
                    ccopy(rsrow[:, o:o + s], ps_r[:, :s])
                rsdram = dram.tile([NI], f32, tag=f"rsdram{h}",
                                   name=f"rsdram{h}")
                nc.sync.dma_start(
                    bass.AP(rsdram.tensor, 0, [[NI, 1], [1, NI]]), rsrow[:])
                rsct = p6sm.tile([128, IT], f32, tag="rsct", name="rsct")
                nc.sync.dma_start(
                    rsct[:], bass.AP(rsdram.tensor, 0, [[1, 128], [128, IT]]))
                rscr = p6sm.tile([128, IT], f32, tag="rscr", name="rscr")
                nc.vector.reciprocal(rscr[:], rsct[:])
                rsdram2 = dram.tile([NI], f32, tag=f"rsdram2{h}",
                                    name=f"rsdram2{h}")
                nc.sync.dma_start(
                    bass.AP(rsdram2.tensor, 0, [[1, 128], [128, IT]]),
                    rscr[:])
                # E blocks [i, j] via PE transpose (bf16)
                es = []
                for it in range(IT):
                    ps_t = pstr.tile([128, N2], bf16, tag="tr", name="tr")
                    for jt in range(JT):
                        nc.tensor.transpose(
                            ps_t[:, jt * 128:(jt + 1) * 128],
                            ets[jt][:, it * 128:(it + 1) * 128], ident[:])
                    e = p6e.tile([128, N2], bf16, tag="e", name="e")
                    ccopy(e[:], ps_t[:])
                    es.append(e)
                rsrow_b = p6sm.tile([DV, NI], f32, tag="rsrow_b",
                                    name="rsrow_b")
                nc.sync.dma_start(
                    rsrow_b[:],
                    bass.AP(rsdram2.tensor, 0, [[0, DV], [1, NI]]))
                # out1 normalized -> DRAM rows [h*96, +96]
                o1h = p6sm.tile([DV, NI], bf16, tag="o1h", name="o1h")
                nc.vector.tensor_mul(o1h[:], ps_o1[:], rsrow_b[:])
                nc.sync.dma_start(
                    bass.AP(o1dram.tensor, h * DV * NI, [[NI, DV], [1, NI]]),
                    o1h[:])
                # out2^T partial [DV, N2]: v1 rows scaled by 1/rs, then
                # contract over i
                ps_o2 = pso2.tile([DV, N2], f32, tag="o2", name="o2")
                for it in range(IT):
                    v1p = p6sm.tile([128, DV], bf16, tag="v1p", name="v1p")
                    nc.vector.tensor_scalar(
                        v1p[:], v1n[:, it, h * DV:(h + 1) * DV],
                        rscr[:, it:it + 1], None, ALU.mult)
                    nc.tensor.matmul(ps_o2[:], v1p[:], es[it][:],
                                     start=(it == 0), stop=(it == IT - 1))
                o2h = p6sm.tile([DV, N2], f32, tag="o2h", name="o2h")
                ccopy(o2h[:], ps_o2[:])
                nc.sync.dma_start(ar_in[h * DV:(h + 1) * DV, :], o2h[:])
        attp_cm.__exit__(None, None, None)

        # ---- P7: pair AllReduce of out2 partials ----
        nc.gpsimd.collective_compute(
            "AllReduce", ALU.add,
            replica_groups=[[0, 1], [2, 3], [4, 5], [6, 7]],
            ins=[ar_in[:].opt()], outs=[ar_out[:].opt()])

        # ---- P8: x2 + residual + FFN-x, per i-half ----
        with tc.tile_pool(name="p8w", bufs=1) as p8w, \
             tc.tile_pool(name="p8", bufs=1) as p8, \
             tc.tile_pool(name="p8h", bufs=2) as p8h, \
             tc.tile_pool(name="p8s", bufs=3) as p8s, \
             tc.tile_pool(name="p8w1", bufs=3) as p8w1, \
             tc.tile_pool(name="p8w2", bufs=4) as p8w2, \
             tc.tile_pool(name="p8ps", bufs=2, space="PSUM") as p8ps, \
             tc.tile_pool(name="p8psb", bufs=2, space="PSUM") as p8psb:
            Wo1 = chunked(p8w, "Wo1")
            for half in range(2):
                o = half * 512
                xTh = p8h.tile([128, CD, 512], f32r, tag="xTh", name="xTh")
                nc.sync.dma_start(
                    xTh[:], d_in["xT"].ap()[:, o:o + 512]
                    .rearrange("(c p) n -> p c n", p=128))
                o1c = p8h.tile([128, CD, 512], bf16, tag="o1c", name="o1c")
                nc.sync.dma_start(
                    o1c[:],
                    o1dram[:].rearrange("(c p n) -> p c n",
                                        p=128, n=NI)[:, :, o:o + 512])
                x4T = p8.tile([128, CD, 512], f32r, tag="x4T", name="x4T")
                for mi in range(CD):
                    ps = p8ps.tile([128, 512], f32, tag="x2ps", name="x2ps")
                    mm(ps[:], [(Wo1[:, ce, mi * 128:(mi + 1) * 128],
                                o1c[:, ce, :]) for ce in range(CD)])
                    nc.vector.scalar_tensor_tensor(
                        x4T[:, mi, :], ps[:], vt["bo1"][:, mi, :],
                        xTh[:, mi, :], ALU.add, ALU.add)
                negmu, rstd = stats_of(x4T, CD, 512, p8ps, p8, "s4")
                x4ln = p8.tile([128, CD, 512], bf16, tag="x4ln", name="x4ln")
                normalize(x4T, x4ln, CD, negmu, rstd, vt["fx_g"], vt["fx_b"])
                h1 = p8.tile([128, CF, 512], bf16, tag="h1", name="h1")
                for fc in range(CF):
                    w1c = p8w1.tile([128, CD, 128], bf16, tag="w1c",
                                    name="w1c")
                    nc.sync.dma_start(
                        w1c[:], d_in["fx_w1"].ap()[:, fc * 128:(fc + 1) * 128]
                        .rearrange("(c p) n -> p c n", p=128))
                    ps_h = p8psb.tile([128, 512], f32, tag="hps", name="hps")
                    mm(ps_h[:], [(w1c[:, ce, :], x4ln[:, ce, :])
                                 for ce in range(CD)])
                    nc.scalar.activation(h1[:, fc, :], ps_h[:], AF.Relu,
                                         bias=vt["fx_b1"][:, fc, :])
                for mi in range(CD):
                    w2c = p8w2.tile([128, CF, 128], bf16, tag="w2c",
                                    name="w2c")
                    nc.sync.dma_start(
                        w2c[:],
                        d_in["fx_w2"].ap()[:, mi * 128:(mi + 1) * 128]
                        .rearrange("(c p) n -> p c n", p=128))
                    ps = p8psb.tile([128, 512], f32, tag="x5ps", name="x5ps")
                    for fc in range(CF):
                        nc.tensor.matmul(ps[:], w2c[:, fc, :], h1[:, fc, :],
                                         start=(fc == 0), stop=(fc == CF - 1))
                    x5s = p8s.tile([128, 512], f32, tag="x5s", name="x5s")
                    nc.vector.scalar_tensor_tensor(
                        x5s[:], ps[:], vt["fx_b2"][:, mi, :],
                        x4T[:, mi, :], ALU.add, ALU.add)
                    nc.sync.dma_start(
                        x5T_d.ap()[mi * 128:(mi + 1) * 128, o:o + 512],
                        x5s[:])

        # ---- P9: y2 + residual + FFN-y (full width, duplicated in pair) ----
        with tc.tile_pool(name="p9w", bufs=1) as p9w, \
             tc.tile_pool(name="p9", bufs=1) as p9, \
             tc.tile_pool(name="p9s", bufs=3) as p9s, \
             tc.tile_pool(name="p9w2", bufs=4) as p9w2, \
             tc.tile_pool(name="p9ps", bufs=2, space="PSUM") as p9ps, \
             tc.tile_pool(name="p9psb", bufs=2, space="PSUM") as p9psb:
            Wo2 = chunked(p9w, "Wo2")
            fy_w1 = chunked(p9w, "fy_w1")
            o2r = p9.tile([128, CD, N2], bf16, tag="o2r", name="o2r")
            nc.gpsimd.dma_start(
                out=o2r[:], in_=ar_out[:].rearrange("(c p) n -> p c n", p=128))
            yTr = p9.tile([128, CD, N2], f32r, tag="yTr", name="yTr")
            nc.sync.dma_start(yTr[:],
                              ydram[:].rearrange("(c p n) -> p c n",
                                                 p=128, n=N2))
            y4T = p9.tile([128, CD, N2], f32r, tag="y4T", name="y4T")
            for mi in range(CD):
                ps = p9ps.tile([128, 512], f32, tag="y2ps", name="y2ps")
                mm(ps[:], [(Wo2[:, ce, mi * 128:(mi + 1) * 128],
                            o2r[:, ce, :]) for ce in range(CD)])
                nc.vector.scalar_tensor_tensor(
                    y4T[:, mi, :], ps[:], vt["bo2"][:, mi, :],
                    yTr[:, mi, :], ALU.add, ALU.add)
            negmu, rstd = stats_of(y4T, CD, N2, p9ps, p9, "s5")
            y4ln = p9.tile([128, CD, N2], bf16, tag="y4ln", name="y4ln")
            normalize(y4T, y4ln, CD, negmu, rstd, vt["fy_g"], vt["fy_b"])
            h1y = p9.tile([128, CF, N2], bf16, tag="h1y", name="h1y")
            for fc in range(CF):
                ps_h = p9psb.tile([128, 512], f32, tag="hyps", name="hyps")
                mm(ps_h[:], [(fy_w1[:, ce, fc * 128:(fc + 1) * 128],
                              y4ln[:, ce, :]) for ce in range(CD)])
                nc.scalar.activation(h1y[:, fc, :], ps_h[:], AF.Relu,
                                     bias=vt["fy_b1"][:, fc, :])
            for mi in range(CD):
                w2c = p9w2.tile([128, CF, 128], bf16, tag="yw2c", name="yw2c")
                nc.sync.dma_start(
                    w2c[:],
                    d_in["fy_w2"].ap()[:, mi * 128:(mi + 1) * 128]
                    .rearrange("(c p) n -> p c n", p=128))
                ps = p9psb.tile([128, 512], f32, tag="y5ps", name="y5ps")
                for fc in range(CF):
                    nc.tensor.matmul(ps[:], w2c[:, fc, :], h1y[:, fc, :],
                                     start=(fc == 0), stop=(fc == CF - 1))
                y5s = p9s.tile([128, 512], f32, tag="y5s", name="y5s")
                nc.vector.scalar_tensor_tensor(
                    y5s[:], ps[:], vt["fy_b2"][:, mi, :],
                    y4T[:, mi, :], ALU.add, ALU.add)
                nc.sync.dma_start(y5T_d.ap()[mi * 128:(mi + 1) * 128, :],
                                  y5s[:])


_CACHE = {}
LAST_RESULT = None


def kernel(**inputs):
    if "nc" not in _CACHE:
        _CACHE["nc"] = build_bass()
        _CACHE["pos"] = _positional_embed()
    nc = _CACHE["nc"]
    pos = _CACHE["pos"]

    f = {k: np.ascontiguousarray(np.asarray(v, dtype=np.float32))
         for k, v in inputs.items()}
    col = lambda a: np.ascontiguousarray(a.reshape(-1, 1))
    bf = lambda a: np.ascontiguousarray(a.astype(ml_dtypes.bfloat16))
    shared = dict(
        W_res=f["W_res"], Wq=bf(f["Wq"]), Wk=bf(f["Wk"]), Wv1=bf(f["Wv1"]),
        Wv2=bf(f["Wv2"]),
        Wo1=bf(f["Wo1"]), Wo2=bf(f["Wo2"]), Wrel=bf(f["Wrel"]),
        fx_w1=bf(f["fx_w1"]), fx_w2=bf(f["fx_w2"]), fy_w1=bf(f["fy_w1"]),
        fy_w2=bf(f["fy_w2"]),
        lnx_g=col(f["lnx_g"]), lnx_b=col(f["lnx_b"]),
        lny_g=col(f["lny_g"]), lny_b=col(f["lny_b"]),
        fx_g=col(f["fx_g"]), fx_b=col(f["fx_b"]),
        fy_g=col(f["fy_g"]), fy_b=col(f["fy_b"]),
        bo1=col(f["bo1"]), bo2=col(f["bo2"]),
        fx_b2=col(f["fx_b2"]), fy_b2=col(f["fy_b2"]),
        fx_b1=col(f["fx_b1"]), fy_b1=col(f["fy_b1"]),
        relb=col(f["rel_pos_bias"]),
    )
    in_maps = []
    for c in range(NCORES):
        b, ih = divmod(c, 2)
        i0 = ih * NI
        m = dict(shared)
        m["xT"] = np.ascontiguousarray(f["x"][b, i0:i0 + NI, :].T)
        m["y0T"] = np.ascontiguousarray(f["y0"][b].T)
        m["posT"] = bf(pos[i0:i0 + LREL].T)
        in_maps.append(m)

    import os
    kwargs = {}
    if os.environ.get("KERNEL_TRACE"):
        kwargs = dict(trace=True,
                      trace_cores=[int(v) for v in
                                   os.environ.get("KERNEL_TRACE_CORES",
                                                  "0").split(",")])
    res = bass_utils.run_bass_kernel_spmd(nc, in_maps,
                                          core_ids=list(range(NCORES)),
                                          **kwargs)
    global LAST_RESULT
    LAST_RESULT = res
    x5 = np.empty((B, N1, D), np.float32)
    y5 = np.empty((B, N2, D), np.float32)
    for c in range(NCORES):
        b, ih = divmod(c, 2)
        x5[b, ih * NI:(ih + 1) * NI, :] = res.results[c]["x5T"].T
        if ih == 0:
            y5[b] = res.results[c]["y5T"].T
    return x5, y5


# revision 26
# speedup vs baseline: 1.6253x; 1.0616x over previous
"""CrossFormer layer (nn_CrossFormerLayer) on 8 trn2 NeuronCores.

Sharding: core c -> batch b = c//2, i-half ih = c%2 (1024 of the 2048
n1 rows). The x-path is fully local to a core; the y-path needs one
pair-wise (2-core) AllReduce of the partial out2 = attn^T @ v1 (the
contraction over i is split across the pair). The y tail
(out2 @ Wo2 + FFN) is duplicated inside each pair (cheap).

All activations are kept in transposed [feature, seq] layout so every
matmul chains without activation transposes. The attention matrix E
(= exp(logits), softmax numerator) is the one tensor needed in both
orientations; it is PE-transposed per 128x128 block. Softmax skips the
max-subtraction (logits are O(1) by construction). The Transformer-XL
relative shift is free: rel is stored [j, m] with row pitch 1535 in
DRAM and the shifted matrix is a strided read
shifted[j, i] = flat[j*1534 + 511 + i].
"""
import math
import ml_dtypes
import numpy as np

import concourse.bacc as bacc
import concourse.bass as bass
import concourse.mybir as mybir
import concourse.tile as tile
from concourse import bass_utils
from concourse.masks import make_identity

f32 = mybir.dt.float32
f32r = mybir.dt.float32r
bf16 = mybir.dt.bfloat16
AF = mybir.ActivationFunctionType
ALU = mybir.AluOpType

B, N1, N2 = 4, 2048, 512
D, H, DK, DV, NRP = 768, 8, 64, 96, 96
DY0 = 1536
DFF = 2 * D
NI = N1 // 2              # 1024 i-rows per core
LREL = N2 + NI            # 1536 rel columns (local m window, padded even)
EPS = 1e-5
SCALE = DK ** -0.5
NCORES = 8
CD = D // 128             # 6
CY = DY0 // 128           # 12
CHD = (H * DK) // 128     # 4
CF = DFF // 128           # 12
JT = N2 // 128            # 4
IT = NI // 128            # 8


def _positional_embed():
    """Enformer relative positional features, pure numpy (fp64->fp32)."""
    n1, fs = N1, NRP
    d = np.arange(-n1 + 1, n1)
    ad = np.abs(d).astype(np.float64)[:, None]
    nb = fs // 6
    max_range = math.log(n1) / math.log(2.0)
    half_life = 2.0 ** np.linspace(3.0, max_range, nb)
    f_exp = np.exp(-math.log(2.0) / half_life * ad)
    cw = 2.0 ** np.arange(1, nb + 1).astype(np.float64) - 1.0
    f_cm = (cw > ad).astype(np.float64)
    stddev = n1 / (2.0 * nb)
    start_mean = n1 / nb
    mean = np.linspace(start_mean, float(n1), nb)
    conc = (mean / stddev) ** 2
    rate = mean / (stddev ** 2)
    lgamma = np.array([math.lgamma(c) for c in conc])
    ad_safe = np.where(ad > 0, ad, 1.0)
    logp = (conc - 1.0) * np.log(ad_safe) - rate * ad - (lgamma - conc * np.log(rate))
    logp = np.where(ad > 0, logp, -np.inf)
    prob = np.exp(logp) + 1e-8
    f_g = prob / np.max(prob, -1, keepdims=True)
    emb = np.concatenate([f_exp, f_cm, f_g], -1)
    sign = np.sign(d).astype(np.float64)[:, None]
    return np.concatenate([emb, sign * emb], -1).astype(np.float32)  # [4095, 96]


def _nsplits(n):
    out, o = [], 0
    while o < n:
        s = min(512, n - o)
        out.append((o, s))
        o += s
    return out


def build_bass():
    nc = bacc.Bacc("TRN2", target_bir_lowering=False, debug=False,
                   num_devices=NCORES)

    F32R_INPUTS = {"xT", "y0T", "W_res"}
    BF16_INPUTS = {"posT", "Wq", "Wk", "Wv1", "Wv2", "Wo1", "Wo2", "Wrel",
                   "fx_w1", "fx_w2", "fy_w1", "fy_w2"}

    def din(name, shape):
        dt = (f32r if name in F32R_INPUTS
              else bf16 if name in BF16_INPUTS else f32)
        return nc.dram_tensor(name, shape, dt, kind="ExternalInput")

    d_in = {}
    for nm, shape in [
        ("xT", [D, NI]), ("y0T", [DY0, N2]), ("posT", [NRP, LREL]),
        ("W_res", [DY0, D]), ("Wq", [D, H * DK]), ("Wk", [D, H * DK]),
        ("Wv1", [D, H * DV]), ("Wv2", [D, H * DV]),
        ("Wo1", [H * DV, D]), ("Wo2", [H * DV, D]), ("Wrel", [NRP, H * DK]),
        ("fx_w1", [D, DFF]), ("fx_w2", [DFF, D]),
        ("fy_w1", [D, DFF]), ("fy_w2", [DFF, D]),
        ("lnx_g", [D, 1]), ("lnx_b", [D, 1]), ("lny_g", [D, 1]),
        ("lny_b", [D, 1]), ("fx_g", [D, 1]), ("fx_b", [D, 1]),
        ("fy_g", [D, 1]), ("fy_b", [D, 1]), ("bo1", [D, 1]), ("bo2", [D, 1]),
        ("fx_b2", [D, 1]), ("fy_b2", [D, 1]), ("fx_b1", [DFF, 1]),
        ("fy_b1", [DFF, 1]), ("relb", [H * DK, 1]),
    ]:
        d_in[nm] = din(nm, shape)
    x5T_d = nc.dram_tensor("x5T", [D, NI], f32, kind="ExternalOutput")
    y5T_d = nc.dram_tensor("y5T", [D, N2], f32, kind="ExternalOutput")

    with tile.TileContext(nc) as tc:
        _build(nc, tc, d_in, x5T_d, y5T_d)
    nc.compile()
    return nc


def _build(nc, tc, d_in, x5T_d, y5T_d):
    def mm(ps_ap, pairs):
        n = len(pairs)
        for i, (l, r) in enumerate(pairs):
            nc.tensor.matmul(ps_ap, l, r, start=(i == 0), stop=(i == n - 1))

    def chunked(pool, name, tag=None):
        d = d_in[name]
        rows, cols = d.shape
        c = (rows + 127) // 128
        t = pool.tile([128, c, cols], d.dtype, tag=(tag or name),
                      name=(tag or name))
        nc.sync.dma_start(t[:], d.ap().rearrange("(c p) n -> p c n", p=128))
        return t

    ccopy = nc.scalar.copy

    with tc.tile_pool(name="const", bufs=1) as cpool, \
         tc.tile_pool(name="dram", bufs=1, space="DRAM") as dram:

        stage32 = cpool.tile([128, 128], f32)
        ones128 = cpool.tile([128, 128], f32r)
        nc.vector.memset(stage32[:], 1.0)
        ccopy(ones128[:], stage32[:])
        ones_bf = cpool.tile([128, 1], bf16)
        ccopy(ones_bf[:], stage32[:, 0:1])
        ident32 = cpool.tile([128, 128], f32)
        make_identity(nc, ident32[:])
        ident = cpool.tile([128, 128], bf16)
        ccopy(ident[:], ident32[:])
        epst = cpool.tile([128, 1], f32)
        nc.vector.memset(epst[:], EPS)
        vt = {nm: chunked(cpool, nm) for nm in
              ["lnx_g", "lnx_b", "lny_g", "lny_b", "fx_g", "fx_b", "fy_g",
               "fy_b", "bo1", "bo2", "fx_b2", "fy_b2", "fx_b1", "fy_b1",
               "relb"]}

        ydram = dram.tile([D * N2], f32r, name="ydram")
        o1dram = dram.tile([H * DV * NI], bf16, name="o1dram")

        def stats_of(tT, C, N, pspool, spool, tag):
            """LN stats over the feature dim (C*128). The all-ones [128,128]
            lhsT replicates column sums to every partition, so the stats come
            out already broadcast: returns (negmu_b, rstd_b) [128, N]."""
            nfeat = C * 128
            negmu_b = spool.tile([128, N], f32, tag=tag + "_mub",
                                 name=tag + "_mub")
            rstd_b = spool.tile([128, N], f32, tag=tag + "_rsb",
                                name=tag + "_rsb")
            tmp = spool.tile([128, N], f32, tag=tag + "_tmp",
                             name=tag + "_tmp")
            for (o, s) in _nsplits(N):
                ps_s = pspool.tile([128, 512], f32, tag="stat_ps",
                                   name="stat_ps")
                mm(ps_s[:, :s],
                   [(ones128[:], tT[:, ci, o:o + s]) for ci in range(C)])
                nc.scalar.mul(negmu_b[:, o:o + s], ps_s[:, :s], -1.0 / nfeat)
                ps_q = pspool.tile([128, 512], f32, tag="stat_ps",
                                   name="stat_ps")
                for ci in range(C):
                    sq = spool.tile([128, 512], f32r, tag=tag + "_sqb",
                                    name=tag + "_sqb")
                    nc.scalar.square(sq[:, :s], tT[:, ci, o:o + s])
                    nc.tensor.matmul(ps_q[:, :s], ones128[:], sq[:, :s],
                                     start=(ci == 0), stop=(ci == C - 1))
                nc.scalar.mul(tmp[:, o:o + s], ps_q[:, :s], 1.0 / nfeat)
            nc.vector.tensor_mul(rstd_b[:], negmu_b[:], negmu_b[:])
            nc.vector.tensor_sub(tmp[:], tmp[:], rstd_b[:])
            nc.scalar.activation(tmp[:], tmp[:], AF.Sqrt, bias=epst[:, 0:1])
            nc.vector.reciprocal(rstd_b[:], tmp[:])
            return negmu_b, rstd_b

        def normalize(tT, out, C, negmu_b, rstd_b, g, b):
            for ci in range(C):
                nc.vector.tensor_add(out[:, ci, :], tT[:, ci, :], negmu_b[:])
                nc.vector.tensor_mul(out[:, ci, :], out[:, ci, :], rstd_b[:])
                nc.vector.tensor_scalar(out[:, ci, :], out[:, ci, :],
                                        g[:, ci, :], b[:, ci, :],
                                        ALU.mult, ALU.add)

        def proj(pspool, out, W, aT, CE, CM, NA, evict):
            for mi in range(CM):
                for (o, s) in _nsplits(NA):
                    ps = pspool.tile([128, 512], f32, tag="proj_ps",
                                     name="proj_ps")
                    mm(ps[:, :s], [(W[:, ce, mi * 128:(mi + 1) * 128],
                                    aT[:, ce, o:o + s]) for ce in range(CE)])
                    evict(out[:, mi, o:o + s], ps[:, :s])

        # ---- P1: yT = (y0 @ W_res)^T -> DRAM scratch ----
        with tc.tile_pool(name="p1", bufs=1) as p1, \
             tc.tile_pool(name="p1e", bufs=3) as p1e, \
             tc.tile_pool(name="p1ps", bufs=2, space="PSUM") as p1ps:
            W_res = chunked(p1, "W_res")
            y0T = chunked(p1, "y0T")
            for mi in range(CD):
                ps = p1ps.tile([128, 512], f32, tag="proj_ps", name="proj_ps")
                mm(ps[:], [(W_res[:, ce, mi * 128:(mi + 1) * 128],
                            y0T[:, ce, :]) for ce in range(CY)])
                ysb = p1e.tile([128, N2], f32r, tag="ysb", name="ysb")
                ccopy(ysb[:], ps[:])
                nc.sync.dma_start(
                    bass.AP(ydram.tensor, mi * 128 * N2, [[N2, 128], [1, N2]]),
                    ysb[:])

        # ---- P2-P4: layernorms + projections (outputs live into P6) ----
        attp_cm = tc.tile_pool(name="attp", bufs=1)
        attp = attp_cm.__enter__()
        qT = attp.tile([128, CHD, NI], bf16, tag="qT")
        kT = attp.tile([128, CHD, N2], bf16, tag="kT")
        kbT = attp.tile([128, CHD, N2], bf16, tag="kbT")
        v2n = attp.tile([128, JT, H * DV], bf16, tag="v2n")
        v1n = attp.tile([128, IT, H * DV], bf16, tag="v1n")
        relqT = attp.tile([128, CHD, LREL], bf16, tag="relqT")

        # (a)+(b): y layernorm; kT, v2n
        with tc.tile_pool(name="py", bufs=1) as py, \
             tc.tile_pool(name="pyps", bufs=2, space="PSUM") as pyps:
            yT = py.tile([128, CD, N2], f32r, tag="yT")
            nc.sync.dma_start(yT[:],
                              ydram[:].rearrange("(c p n) -> p c n",
                                                 p=128, n=N2))
            negmu_y, rstd_y = stats_of(yT, CD, N2, pyps, py, "sy")
            y1T = py.tile([128, CD, N2], bf16, tag="y1T")
            normalize(yT, y1T, CD, negmu_y, rstd_y, vt["lny_g"], vt["lny_b"])
            Wk = chunked(py, "Wk")
            Wv2 = chunked(py, "Wv2")
            proj(pyps, kT, Wk, y1T, CD, CHD, N2, ccopy)
            proj(pyps, v2n, y1T, Wv2, CD, JT, H * DV, ccopy)
            for ci in range(CHD):
                nc.vector.tensor_scalar(kbT[:, ci, :], kT[:, ci, :],
                                        SCALE, vt["relb"][:, ci, :],
                                        ALU.mult, ALU.add)

        # rel_qT from pos
        with tc.tile_pool(name="pr", bufs=1) as pr, \
             tc.tile_pool(name="prps", bufs=2, space="PSUM") as prps:
            posT = pr.tile([NRP, LREL], bf16, tag="posT")
            nc.sync.dma_start(posT[:], d_in["posT"].ap())
            Wrel = pr.tile([NRP, H * DK], bf16, tag="Wrel")
            nc.sync.dma_start(Wrel[:], d_in["Wrel"].ap())
            for mi in range(CHD):
                for (o, s) in _nsplits(LREL):
                    ps = prps.tile([128, 512], f32, tag="proj_ps",
                                   name="proj_ps")
                    mm(ps[:, :s], [(Wrel[:, mi * 128:(mi + 1) * 128],
                                    posT[:, o:o + s])])
                    ccopy(relqT[:, mi, o:o + s], ps[:, :s])

        # (c)+(d): x layernorm (per i-half); qT, v1n
        with tc.tile_pool(name="px", bufs=1) as px, \
             tc.tile_pool(name="pxps", bufs=2, space="PSUM") as pxps:
            xTt = chunked(px, "xT")
            x1T = px.tile([128, CD, NI], bf16, tag="x1T")
            for half in range(2):
                o = half * 512
                xh = xTt[:, :, o:o + 512]
                negmu_x, rstd_x = stats_of(xh, CD, 512, pxps, px, "sx")
                normalize(xh, x1T[:, :, o:o + 512], CD, negmu_x, rstd_x,
                          vt["lnx_g"], vt["lnx_b"])
            Wq = chunked(px, "Wq")
            Wv1 = chunked(px, "Wv1")
            proj(pxps, qT, Wq, x1T, CD, CHD, NI,
                 lambda o, p: nc.scalar.mul(o, p, SCALE))
            proj(pxps, v1n, x1T, Wv1, CD, IT, H * DV, ccopy)

        # ---- P5: rel[j, m] per head -> DRAM (row pitch LREL) ----
        relbufs = [dram.tile([N2 * LREL], bf16, tag=f"relbuf{h}",
                             name=f"relbuf{h}")
                   for h in range(H)]
        with tc.tile_pool(name="p5", bufs=3) as p5, \
             tc.tile_pool(name="p5ps", bufs=2, space="PSUM") as p5ps:
            for h in range(H):
                hc, ho = h // 2, (h % 2) * DK
                for jt in range(JT):
                    ps = p5ps.tile([128, LREL], f32, tag="rel_ps",
                                   name="rel_ps")
                    kb_l = kbT[:, hc, jt * 128:(jt + 1) * 128][ho:ho + DK, :]
                    for (o, s) in _nsplits(LREL):
                        nc.tensor.matmul(
                            ps[:, o:o + s], kb_l,
                            relqT[:, hc, o:o + s][ho:ho + DK, :],
                            start=True, stop=True)
                    sb = p5.tile([128, LREL], bf16, tag="rel_sb", name="rel_sb")
                    if jt % 2 == 0:
                        ccopy(sb[:], ps[:])
                    else:
                        nc.vector.tensor_copy(sb[:], ps[:])
                    nc.sync.dma_start(
                        bass.AP(relbufs[h].tensor, jt * 128 * LREL,
                                [[LREL, 128], [1, LREL]]),
                        sb[:])

        # ---- P6: attention head loop ----
        ar_in = dram.tile([H * DV, N2], f32, name="ar_in")
        ar_out = dram.tile([H * DV, N2], f32, name="ar_out")

        with tc.tile_pool(name="p6relw", bufs=4) as p6relw, \
             tc.tile_pool(name="p6et", bufs=8) as p6et, \
             tc.tile_pool(name="p6e", bufs=6) as p6e, \
             tc.tile_pool(name="p6sm", bufs=3) as p6sm, \
             tc.tile_pool(name="psct", bufs=2, space="PSUM") as psct, \
             tc.tile_pool(name="pstr", bufs=2, space="PSUM") as pstr, \
             tc.tile_pool(name="psrs", bufs=1, space="PSUM") as psrs, \
             tc.tile_pool(name="pso1", bufs=1, space="PSUM") as pso1, \
             tc.tile_pool(name="pso2", bufs=1, space="PSUM") as pso2:
            for h in range(H):
                hc, ho = h // 2, (h % 2) * DK
                # E^T = exp(content^T + rel_shift) in [j, i] layout
                ets = []
                for jt in range(JT):
                    relw = p6relw.tile([128, NI], bf16, tag="relw",
                                       name="relw")
                    nc.sync.dma_start(
                        relw[:],
                        bass.AP(relbufs[h].tensor,
                                jt * 128 * (LREL - 1) + (N2 - 1),
                                [[LREL - 1, 128], [1, NI]]))
                    et = p6et.tile([128, NI], bf16, tag="et", name="et")
                    k_l = kT[:, hc, jt * 128:(jt + 1) * 128][ho:ho + DK, :]
                    for (o, s) in _nsplits(NI):
                        ps = psct.tile([128, 512], f32, tag="ct", name="ct")
                        nc.tensor.matmul(ps[:, :s], k_l,
                                         qT[:, hc, o:o + s][ho:ho + DK, :],
                                         start=True, stop=True)
                        nc.vector.tensor_tensor(ps[:, :s], ps[:, :s],
                                                relw[:, o:o + s], ALU.add)
                        nc.scalar.activation(et[:, o:o + s], ps[:, :s],
                                             AF.Exp)
                    ets.append(et)
                # out1^T (unnormalized) [DV, NI]
                ps_o1 = pso1.tile([DV, NI], f32, tag="o1", name="o1")
                for (o, s) in _nsplits(NI):
                    for jt in range(JT):
                        nc.tensor.matmul(
                            ps_o1[:, o:o + s],
                            v2n[:, jt, h * DV:(h + 1) * DV],
                            ets[jt][:, o:o + s],
                            start=(jt == 0), stop=(jt == JT - 1))
                # row-sums 1/rs: ones-matmul over ET (j on partitions)
                rsrow = p6sm.tile([1, NI], f32, tag="rsrow", name="rsrow")
                for (o, s) in _nsplits(NI):
                    ps_r = psrs.tile([1, 512], f32, tag="rs", name="rs")
                    for jt in range(JT):
                        nc.tensor.matmul(ps_r[:, :s], ones_bf[:, 0:1],
                                         ets[jt][:, o:o + s],
                                         start=(jt == 0), stop=(jt == JT - 1))
                    ccopy(rsrow[:, o:o + s], ps_r[:, :s])
                rsdram = dram.tile([NI], f32, tag=f"rsdram{h}",
                                   name=f"rsdram{h}")
                nc.sync.dma_start(
                    bass.AP(rsdram.tensor, 0, [[NI, 1], [1, NI]]), rsrow[:])
                rsct = p6sm.tile([128, IT], f32, tag="rsct", name="rsct")
                nc.sync.dma_start(
                    rsct[:], bass.AP(rsdram.tensor, 0, [[1, 128], [128, IT]]))
                rscr = p6sm.tile([128, IT], f32, tag="rscr", name="rscr")
                nc.vector.reciprocal(rscr[:], rsct[:])
                rsdram2 = dram.tile([NI], f32, tag=f"rsdram2{h}",
                                    name=f"rsdram2{h}")
                nc.sync.dma_start(
                    bass.AP(rsdram2.tensor, 0, [[1, 128], [128, IT]]),
                    rscr[:])
                # E blocks [i, j] via PE transpose (bf16)
                es = []
                for it in range(IT):
                    ps_t = pstr.tile([128, N2], bf16, tag="tr", name="tr")
                    for jt in range(JT):
                        nc.tensor.transpose(
                            ps_t[:, jt * 128:(jt + 1) * 128],
                            ets[jt][:, it * 128:(it + 1) * 128], ident[:])
                    e = p6e.tile([128, N2], bf16, tag="e", name="e")
                    ccopy(e[:], ps_t[:])
                    es.append(e)
                rsrow_b = p6sm.tile([DV, NI], f32, tag="rsrow_b",
                                    name="rsrow_b")
                nc.sync.dma_start(
                    rsrow_b[:],
                    bass.AP(rsdram2.tensor, 0, [[0, DV], [1, NI]]))
                # out1 normalized -> DRAM rows [h*96, +96]
                o1h = p6sm.tile([DV, NI], bf16, tag="o1h", name="o1h")
                nc.vector.tensor_mul(o1h[:], ps_o1[:], rsrow_b[:])
                nc.sync.dma_start(
                    bass.AP(o1dram.tensor, h * DV * NI, [[NI, DV], [1, NI]]),
                    o1h[:])
                # out2^T partial [DV, N2]: v1 rows scaled by 1/rs, then
                # contract over i
                ps_o2 = pso2.tile([DV, N2], f32, tag="o2", name="o2")
                for it in range(IT):
                    v1p = p6sm.tile([128, DV], bf16, tag="v1p", name="v1p")
                    nc.vector.tensor_scalar(
                        v1p[:], v1n[:, it, h * DV:(h + 1) * DV],
                        rscr[:, it:it + 1], None, ALU.mult)
                    nc.tensor.matmul(ps_o2[:], v1p[:], es[it][:],
                                     start=(it == 0), stop=(it == IT - 1))
                o2h = p6sm.tile([DV, N2], f32, tag="o2h", name="o2h")
                ccopy(o2h[:], ps_o2[:])
                nc.sync.dma_start(ar_in[h * DV:(h + 1) * DV, :], o2h[:])
        attp_cm.__exit__(None, None, None)

        # ---- P7: pair AllReduce of out2 partials ----
        nc.gpsimd.collective_compute(
            "AllReduce", ALU.add,
            replica_groups=[[0, 1], [2, 3], [4, 5], [6, 7]],
            ins=[ar_in[:].opt()], outs=[ar_out[:].opt()])

        # ---- P8: x2 + residual + FFN-x, per i-half ----
        with tc.tile_pool(name="p8w", bufs=1) as p8w, \
             tc.tile_pool(name="p8", bufs=1) as p8, \
             tc.tile_pool(name="p8h", bufs=2) as p8h, \
             tc.tile_pool(name="p8s", bufs=3) as p8s, \
             tc.tile_pool(name="p8w1", bufs=3) as p8w1, \
             tc.tile_pool(name="p8w2", bufs=4) as p8w2, \
             tc.tile_pool(name="p8ps", bufs=2, space="PSUM") as p8ps, \
             tc.tile_pool(name="p8psb", bufs=2, space="PSUM") as p8psb:
            Wo1 = chunked(p8w, "Wo1")
            for half in range(2):
                o = half * 512
                xTh = p8h.tile([128, CD, 512], f32r, tag="xTh", name="xTh")
                nc.sync.dma_start(
                    xTh[:], d_in["xT"].ap()[:, o:o + 512]
                    .rearrange("(c p) n -> p c n", p=128))
                o1c = p8h.tile([128, CD, 512], bf16, tag="o1c", name="o1c")
                nc.sync.dma_start(
                    o1c[:],
                    o1dram[:].rearrange("(c p n) -> p c n",
                                        p=128, n=NI)[:, :, o:o + 512])
                x4T = p8.tile([128, CD, 512], f32r, tag="x4T", name="x4T", bufs=2)
                for mi in range(CD):
                    ps = p8ps.tile([128, 512], f32, tag="x2ps", name="x2ps")
                    mm(ps[:], [(Wo1[:, ce, mi * 128:(mi + 1) * 128],
                                o1c[:, ce, :]) for ce in range(CD)])
                    nc.vector.scalar_tensor_tensor(
                        x4T[:, mi, :], ps[:], vt["bo1"][:, mi, :],
                        xTh[:, mi, :], ALU.add, ALU.add)
                negmu, rstd = stats_of(x4T, CD, 512, p8ps, p8, "s4")
                x4ln = p8.tile([128, CD, 512], bf16, tag="x4ln", name="x4ln", bufs=2)
                normalize(x4T, x4ln, CD, negmu, rstd, vt["fx_g"], vt["fx_b"])
                h1 = p8.tile([128, CF, 512], bf16, tag="h1", name="h1", bufs=2)
                for fc in range(CF):
                    w1c = p8w1.tile([128, CD, 128], bf16, tag="w1c",
                                    name="w1c")
                    nc.sync.dma_start(
                        w1c[:], d_in["fx_w1"].ap()[:, fc * 128:(fc + 1) * 128]
                        .rearrange("(c p) n -> p c n", p=128))
                    ps_h = p8psb.tile([128, 512], f32, tag="hps", name="hps")
                    mm(ps_h[:], [(w1c[:, ce, :], x4ln[:, ce, :])
                                 for ce in range(CD)])
                    nc.scalar.activation(h1[:, fc, :], ps_h[:], AF.Relu,
                                         bias=vt["fx_b1"][:, fc, :])
                for mi in range(CD):
                    w2c = p8w2.tile([128, CF, 128], bf16, tag="w2c",
                                    name="w2c")
                    nc.sync.dma_start(
                        w2c[:],
                        d_in["fx_w2"].ap()[:, mi * 128:(mi + 1) * 128]
                        .rearrange("(c p) n -> p c n", p=128))
                    ps = p8psb.tile([128, 512], f32, tag="x5ps", name="x5ps")
                    for fc in range(CF):
                        nc.tensor.matmul(ps[:], w2c[:, fc, :], h1[:, fc, :],
                                         start=(fc == 0), stop=(fc == CF - 1))
                    x5s = p8s.tile([128, 512], f32, tag="x5s", name="x5s")
                    nc.vector.scalar_tensor_tensor(
                        x5s[:], ps[:], vt["fx_b2"][:, mi, :],
                        x4T[:, mi, :], ALU.add, ALU.add)
                    nc.sync.dma_start(
                        x5T_d.ap()[mi * 128:(mi + 1) * 128, o:o + 512],
                        x5s[:])

        # ---- P9: y2 + residual + FFN-y (full width, duplicated in pair) ----
        with tc.tile_pool(name="p9w", bufs=1) as p9w, \
             tc.tile_pool(name="p9", bufs=1) as p9, \
             tc.tile_pool(name="p9s", bufs=3) as p9s, \
             tc.tile_pool(name="p9w2", bufs=4) as p9w2, \
             tc.tile_pool(name="p9ps", bufs=2, space="PSUM") as p9ps, \
             tc.tile_pool(name="p9psb", bufs=2, space="PSUM") as p9psb:
            Wo2 = chunked(p9w, "Wo2")
            fy_w1 = chunked(p9w, "fy_w1")
            o2r = p9.tile([128, CD, N2], bf16, tag="o2r", name="o2r")
            nc.gpsimd.dma_start(
                out=o2r[:], in_=ar_out[:].rearrange("(c p) n -> p c n", p=128))
            yTr = p9.tile([128, CD, N2], f32r, tag="yTr", name="yTr")
            nc.sync.dma_start(yTr[:],
                              ydram[:].rearrange("(c p n) -> p c n",
                                                 p=128, n=N2))
            y4T = p9.tile([128, CD, N2], f32r, tag="y4T", name="y4T")
            for mi in range(CD):
                ps = p9ps.tile([128, 512], f32, tag="y2ps", name="y2ps")
                mm(ps[:], [(Wo2[:, ce, mi * 128:(mi + 1) * 128],
                            o2r[:, ce, :]) for ce in range(CD)])
                nc.vector.scalar_tensor_tensor(
                    y4T[:, mi, :], ps[:], vt["bo2"][:, mi, :],
                    yTr[:, mi, :], ALU.add, ALU.add)
            negmu, rstd = stats_of(y4T, CD, N2, p9ps, p9, "s5")
            y4ln = p9.tile([128, CD, N2], bf16, tag="y4ln", name="y4ln")
            normalize(y4T, y4ln, CD, negmu, rstd, vt["fy_g"], vt["fy_b"])
            h1y = p9.tile([128, CF, N2], bf16, tag="h1y", name="h1y")
            for fc in range(CF):
                ps_h = p9psb.tile([128, 512], f32, tag="hyps", name="hyps")
                mm(ps_h[:], [(fy_w1[:, ce, fc * 128:(fc + 1) * 128],
                              y4ln[:, ce, :]) for ce in range(CD)])
                nc.scalar.activation(h1y[:, fc, :], ps_h[:], AF.Relu,
                                     bias=vt["fy_b1"][:, fc, :])
            for mi in range(CD):
                w2c = p9w2.tile([128, CF, 128], bf16, tag="yw2c", name="yw2c")
                nc.sync.dma_start(
                    w2c[:],
                    d_in["fy_w2"].ap()[:, mi * 128:(mi + 1) * 128]
                    .rearrange("(c p) n -> p c n", p=128))
                ps = p9psb.tile([128, 512], f32, tag="y5ps", name="y5ps")
                for fc in range(CF):
                    nc.tensor.matmul(ps[:], w2c[:, fc, :], h1y[:, fc, :],
                                     start=(fc == 0), stop=(fc == CF - 1))
                y5s = p9s.tile([128, 512], f32, tag="y5s", name="y5s")
                nc.vector.scalar_tensor_tensor(
                    y5s[:], ps[:], vt["fy_b2"][:, mi, :],
                    y4T[:, mi, :], ALU.add, ALU.add)
                nc.sync.dma_start(y5T_d.ap()[mi * 128:(mi + 1) * 128, :],
                                  y5s[:])


_CACHE = {}
LAST_RESULT = None


def kernel(**inputs):
    if "nc" not in _CACHE:
        _CACHE["nc"] = build_bass()
        _CACHE["pos"] = _positional_embed()
    nc = _CACHE["nc"]
    pos = _CACHE["pos"]

    f = {k: np.ascontiguousarray(np.asarray(v, dtype=np.float32))
         for k, v in inputs.items()}
    col = lambda a: np.ascontiguousarray(a.reshape(-1, 1))
    bf = lambda a: np.ascontiguousarray(a.astype(ml_dtypes.bfloat16))
    shared = dict(
        W_res=f["W_res"], Wq=bf(f["Wq"]), Wk=bf(f["Wk"]), Wv1=bf(f["Wv1"]),
        Wv2=bf(f["Wv2"]),
        Wo1=bf(f["Wo1"]), Wo2=bf(f["Wo2"]), Wrel=bf(f["Wrel"]),
        fx_w1=bf(f["fx_w1"]), fx_w2=bf(f["fx_w2"]), fy_w1=bf(f["fy_w1"]),
        fy_w2=bf(f["fy_w2"]),
        lnx_g=col(f["lnx_g"]), lnx_b=col(f["lnx_b"]),
        lny_g=col(f["lny_g"]), lny_b=col(f["lny_b"]),
        fx_g=col(f["fx_g"]), fx_b=col(f["fx_b"]),
        fy_g=col(f["fy_g"]), fy_b=col(f["fy_b"]),
        bo1=col(f["bo1"]), bo2=col(f["bo2"]),
        fx_b2=col(f["fx_b2"]), fy_b2=col(f["fy_b2"]),
        fx_b1=col(f["fx_b1"]), fy_b1=col(f["fy_b1"]),
        relb=col(f["rel_pos_bias"]),
    )
    in_maps = []
    for c in range(NCORES):
        b, ih = divmod(c, 2)
        i0 = ih * NI
        m = dict(shared)
        m["xT"] = np.ascontiguousarray(f["x"][b, i0:i0 + NI, :].T)
        m["y0T"] = np.ascontiguousarray(f["y0"][b].T)
        m["posT"] = bf(pos[i0:i0 + LREL].T)
        in_maps.append(m)

    import os
    kwargs = {}
    if os.environ.get("KERNEL_TRACE"):
        kwargs = dict(trace=True,
                      trace_cores=[int(v) for v in
                                   os.environ.get("KERNEL_TRACE_CORES",
                                                  "0").split(",")])
    res = bass_utils.run_bass_kernel_spmd(nc, in_maps,
                                          core_ids=list(range(NCORES)),
                                          **kwargs)
    global LAST_RESULT
    LAST_RESULT = res
    x5 = np.empty((B, N1, D), np.float32)
    y5 = np.empty((B, N2, D), np.float32)
    for c in range(NCORES):
        b, ih = divmod(c, 2)
        x5[b, ih * NI:(ih + 1) * NI, :] = res.results[c]["x5T"].T
        if ih == 0:
            y5[b] = res.results[c]["y5T"].T
    return x5, y5
